# revision 3
# baseline (speedup 1.0000x reference)
"""Trainium2 Bass kernel for nn_Cls_Loss_42331197670001.

Reference computation (N=128 samples, C=345 classes, A=512 features):
    dataW[n,c,:] = W[c] - W[labels[n]]
    sigma2[n,c]  = Lambda * dataW[n,c] @ Sigma[labels[n]] @ dataW[n,c]^T
    dW_dMean[n,c]= dataW[n,c] . (mean_target-mean_source)[labels[n]]
    aug = y_s + 0.5*sigma2 + Lambda*dW_dMean ;  loss = mean softmax-CE(aug, labels)

Everything depends on the sample n only through its label l, so the heavy
quadratic form is computed once per *unique* label:
    (W_c - W_l) Sigma_l (W_c - W_l)^T = 0.5*d(l,c) - b(l,c) + 0.5*s(l)
    d(l,c) = W_c S_l W_c^T  with S_l = Sigma_l + Sigma_l^T
             <- the only O(C*A*A) term, on device
    b, s, mean-shift, softmax-CE <- tiny, host numpy in float64

Device computes d(l,c) with labels sharded over 8 cores (u_pc per core) in
one of two algebraically equal forms so the PSUM->SBUF reduce splits across
BOTH vector-capable engines (each PSUM element must cross DVE or ACT once;
that readback is the co-bottleneck with the PE):
  'D' labels (DVE):  P = W M_l   (M_l = upper-tri fold of S_l), then
                     d[c] = sum_b P[c,b] W[c,b] via one DVE
                     scalar_tensor_tensor straight out of PSUM (fp32 in1).
  'A' labels (ACT):  Q = W L_l   (L_l = chol(S_l + shift*I), lower-tri), then
                     d[c] = sum_b Q[c,b]^2 - shift*||W_c||^2 via one ACT
                     activation(Square, accum_out) straight out of PSUM
                     (the shift correction is host float64).
Both triangular forms stream 768 fp8-DoubleRow columns per (label, c-tile):
3 c-tiles of 128 classes, contraction over A=512 as two 256-row stationary
pairs.  Matmuls are emitted stationary-major per group of <=4 labels so
LDWEIGHTS pipelines behind dense matmul streams, and the 8 PSUM banks
ping-pong between the PE (current c-tile) and the consumers (previous).
sg DMAs are batched per group ([128, G*1536] contiguous per partition) to
amortize descriptor overhead; memset-fed junk matmuls warm the PE HAM
clock gate while the first DMAs land, and an early 1-element Square primes
the ACT table load off the critical path.
"""

import math
import sys

import numpy as np

try:
    import concourse.bass as bass
except ImportError:  # harness runs from a bare directory
    sys.path.insert(0, "/opt/trn_rl_repo")
    import concourse.bass as bass

import ml_dtypes

import concourse.mybir as mybir
import concourse.tile as tile
from concourse import bacc
from concourse.bass import ts
from concourse.bass_utils import run_bass_kernel_spmd

N_CORES = 8
A = 512          # feature dim
C = 345          # class count
C_PAD = 384      # 3 * 128
C_TILES = 3

W_SCALE = 16.0
S_SCALE = 32.0
SL_SCALE = 64.0
OUT_SCALE_D = W_SCALE * S_SCALE
OUT_SCALE_A = (W_SCALE * SL_SCALE) ** 2

FP8 = mybir.dt.float8e4
BF16 = mybir.dt.bfloat16
F32 = mybir.dt.float32
FP8_NP = ml_dtypes.float8_e4m3

MULT = mybir.AluOpType.mult
DR = mybir.MatmulPerfMode.DoubleRow
SQUARE = mybir.ActivationFunctionType.Square


def make_groups(u_pc: int) -> list[list[int]]:
    """Slot indices grouped for the PSUM ping-pong: a small first group so
    the PE starts early, then groups of 4."""
    if u_pc <= 2:
        return [list(range(u_pc))]
    groups = [[0, 1]]
    i = 2
    while i < u_pc:
        groups.append(list(range(i, min(i + 4, u_pc))))
        i += 4
    return groups


def slot_types(u_pc: int) -> list[str]:
    """'D' (DVE consumes) or 'A' (ACT consumes) per slot.  First half of
    each group D, second half A; the leading group of 2 is all D.  This
    lands near the DVE/ACT throughput balance (~4:3)."""
    types = ["A"] * u_pc
    for g in make_groups(u_pc):
        n = len(g)
        nd = n if n <= 2 else (n + 1) // 2
        for k, s in enumerate(g):
            types[s] = "D" if k < nd else "A"
    return types


def build_nc(u_pc: int) -> bass.Bass:
    """Per core: u_pc labels; dout[p, t, j] = d-value for class c = 128*t + p
    of slot j (scaled by OUT_SCALE_D or OUT_SCALE_A depending on slot type)."""
    groups = make_groups(u_pc)
    types = slot_types(u_pc)

    nc = bacc.Bacc()
    wt8 = nc.dram_tensor("wt8", [128, 4, C_PAD], FP8, kind="ExternalInput")
    w32 = nc.dram_tensor("w32", [128, C_TILES, A], F32, kind="ExternalInput")
    # per slot j: [:, j, :, 0:512]  = 256 contraction rows (a-pair) x all cols
    #            [:, j, :, 512:768] = the other 256 rows x their 256 cols
    sg = nc.dram_tensor("sg", [128, u_pc, 2, 768], FP8, kind="ExternalInput")
    dout = nc.dram_tensor("dout", [128, C_TILES, u_pc], F32, kind="ExternalOutput")

    with tile.TileContext(nc) as tc:
        with (
            tc.tile_pool(name="singles", bufs=1) as singles,
            tc.tile_pool(name="sgp", bufs=3) as sgpool,
            tc.tile_pool(name="scr", bufs=8) as scrpool,
            tc.tile_pool(name="psum", bufs=8, space="PSUM") as ppool,
        ):
            # Junk feed with no DMA dependency: PE warms its HAM clock gate
            # while the real inputs stream in.
            junkf = singles.tile([128, 2, A], FP8)
            nc.gpsimd.memset(junkf[:], 0)
            # Prime the ACT Square table load off the critical path.
            prime = singles.tile([128, 1], F32)
            nc.vector.memset(prime[:], 1.0)
            prime_o = singles.tile([128, 1], F32)
            nc.scalar.activation(out=prime_o[:], in_=prime[:], func=SQUARE)

            wt8_sb = singles.tile([128, 4, C_PAD], FP8)
            nc.sync.dma_start(out=wt8_sb[:], in_=wt8[:])
            w32_sb = singles.tile([128, C_TILES, A], F32)
            nc.sync.dma_start(out=w32_sb[:], in_=w32[:])
            d_all = singles.tile([128, C_TILES, u_pc], F32)

            junkp = ppool.tile([128, A], F32, tag="ps")
            for _ in range(6):
                nc.tensor.matmul(
                    junkp[:], lhsT=junkf[:, :, 0:128], rhs=junkf[:],
                    start=True, stop=True, perf_mode=DR,
                )

            for slots in groups:
                g0, gn = slots[0], len(slots)
                if gn == 2 and g0 == 0:
                    gt = singles.tile([128, gn, 2, 768], FP8)
                elif gn != 4:
                    gt = singles.tile([128, gn, 2, 768], FP8)
                else:
                    gt = sgpool.tile([128, gn, 2, 768], FP8, tag="sg")
                nc.sync.dma_start(out=gt[:], in_=sg[:, g0 : g0 + gn])

                d_loc = [k for k in range(gn) if types[slots[k]] == "D"]
                a_loc = [k for k in range(gn) if types[slots[k]] == "A"]
                for t in range(C_TILES):
                    lhs01 = wt8_sb[:, 0:2, ts(t, 128)]
                    lhs23 = wt8_sb[:, 2:4, ts(t, 128)]
                    ps = [
                        ppool.tile([128, A], F32, tag="ps", name=f"ps{k}")
                        for k in range(gn)
                    ]
                    # pass 1 -- stationary a01: D-labels' first two matmuls
                    # (M rows 0:256 stream cols 0:512)
                    for k in d_loc:
                        nc.tensor.matmul(
                            ps[k][:, 0:256], lhsT=lhs01,
                            rhs=gt[:, k, :, 0:256],
                            start=True, stop=True, perf_mode=DR,
                        )
                        nc.tensor.matmul(
                            ps[k][:, 256:512], lhsT=lhs01,
                            rhs=gt[:, k, :, 256:512],
                            start=True, stop=False, perf_mode=DR,
                        )
                    # pass 2 -- stationary a23: D-labels finish (M rows
                    # 256:512 x cols 256:512); A-labels' first two matmuls
                    # (L rows 256:512 stream cols 0:512)
                    for k in d_loc:
                        nc.tensor.matmul(
                            ps[k][:, 256:512], lhsT=lhs23,
                            rhs=gt[:, k, :, 512:768],
                            start=False, stop=True, perf_mode=DR,
                        )
                    for k in a_loc:
                        nc.tensor.matmul(
                            ps[k][:, 0:256], lhsT=lhs23,
                            rhs=gt[:, k, :, 0:256],
                            start=True, stop=False, perf_mode=DR,
                        )
                        nc.tensor.matmul(
                            ps[k][:, 256:512], lhsT=lhs23,
                            rhs=gt[:, k, :, 256:512],
                            start=True, stop=True, perf_mode=DR,
                        )
                    # pass 3 -- stationary a01 again: A-labels finish
                    # (L rows 0:256 x cols 0:256)
                    for k in a_loc:
                        nc.tensor.matmul(
                            ps[k][:, 0:256], lhsT=lhs01,
                            rhs=gt[:, k, :, 512:768],
                            start=False, stop=True, perf_mode=DR,
                        )
                    # consumers: one engine pass per PSUM tile
                    for k in d_loc:
                        j = slots[k]
                        o = scrpool.tile([128, A], BF16, tag="o")
                        nc.vector.scalar_tensor_tensor(
                            out=o[:], in0=ps[k][:], scalar=1.0,
                            in1=w32_sb[:, t, :], op0=MULT, op1=MULT,
                            accum_out=d_all[:, t, j : j + 1],
                        )
                    for k in a_loc:
                        j = slots[k]
                        o = scrpool.tile([128, A], BF16, tag="oa")
                        nc.scalar.activation(
                            out=o[:], in_=ps[k][:], func=SQUARE,
                            accum_out=d_all[:, t, j : j + 1],
                        )
            nc.sync.dma_start(out=dout[:], in_=d_all[:])
    nc.compile()
    return nc


def _q8(x: np.ndarray) -> np.ndarray:
    return np.clip(x, -240.0, 240.0).astype(FP8_NP)


def _pack_tri(rows_hi: np.ndarray, corner: np.ndarray, scale: float) -> np.ndarray:
    """[256, 512] full-width rows + [256, 256] corner -> [128, 2, 768] fp8
    in DoubleRow layout (contraction row = 128*ko + ki)."""
    out = np.empty((128, 2, 768), FP8_NP)
    out[:, :, 0:512] = _q8(
        rows_hi.reshape(2, 128, A).transpose(1, 0, 2) * scale
    )
    out[:, :, 512:768] = _q8(
        corner.reshape(2, 128, 256).transpose(1, 0, 2) * scale
    )
    return out


def host_pack(fc_weight: np.ndarray, lab_pad: np.ndarray, cov: np.ndarray):
    """Build device inputs.  Returns (wt8, w32, sg[all cores], shifts,
    S_sym) where sg is [n_slots, 128, 2, 768] (slot-major; reshaped to the
    [128, u_pc, 2, 768] per-core layout by the caller)."""
    n_lab = len(lab_pad)
    u_pc = n_lab // N_CORES
    types = slot_types(u_pc)

    w_pad = np.zeros((C_PAD, A), np.float32)
    w_pad[:C] = fc_weight
    wt = np.ascontiguousarray(w_pad.T.reshape(4, 128, C_PAD).transpose(1, 0, 2))
    wt8 = _q8(wt * W_SCALE)
    w32 = np.ascontiguousarray(w_pad.reshape(C_TILES, 128, A).transpose(1, 0, 2))

    sgath = cov[lab_pad].astype(np.float64)
    s_sym = sgath + sgath.transpose(0, 2, 1)
    sig = s_sym.std(axis=(1, 2))
    eye = np.eye(A)

    sg = np.empty((n_lab, 128, 2, 768), FP8_NP)
    shifts = np.zeros(n_lab)
    for i in range(n_lab):
        if types[i % u_pc] == "D":
            m = np.triu(s_sym[i]) + np.triu(s_sym[i], 1)
            sg[i] = _pack_tri(m[0:256, :], m[256:512, 256:512], S_SCALE)
        else:
            c = 1.12 * 2.0 * math.sqrt(A) * sig[i]
            while True:
                try:
                    low = np.linalg.cholesky(s_sym[i] + c * eye)
                    break
                except np.linalg.LinAlgError:
                    c *= 1.15
            shifts[i] = c
            sg[i] = _pack_tri(low[256:512, :], low[0:256, 0:256], SL_SCALE)
    return wt8, w32, sg, shifts, s_sym


_NC_CACHE: dict[int, bass.Bass] = {}


def _device_dS(fc_weight, uniq, cov):
    """Run the Bass kernel on 8 cores; returns (d_S [U, C] float64, S_sym)."""
    U = len(uniq)
    u_pc = math.ceil(U / N_CORES)
    u_pad = u_pc * N_CORES
    lab_pad = np.concatenate([uniq, np.full(u_pad - U, uniq[0], dtype=uniq.dtype)])
    wt8, w32, sg, shifts, s_sym = host_pack(fc_weight, lab_pad, cov)

    if u_pc not in _NC_CACHE:
        _NC_CACHE[u_pc] = build_nc(u_pc)
    nc = _NC_CACHE[u_pc]

    in_maps = [
        {
            "wt8": wt8,
            "w32": w32,
            # [u_pc, 128, 2, 768] -> [128, u_pc, 2, 768]
            "sg": np.ascontiguousarray(
                sg[i * u_pc : (i + 1) * u_pc].transpose(1, 0, 2, 3)
            ),
        }
        for i in range(N_CORES)
    ]
    res = run_bass_kernel_spmd(nc, in_maps, core_ids=list(range(N_CORES)))

    types = slot_types(u_pc)
    wn2 = (fc_weight.astype(np.float64) ** 2).sum(axis=1)  # [C]
    wn2_pad = np.zeros(C_PAD)
    wn2_pad[:C] = wn2
    d_rows = []
    for i, r in enumerate(res.results):
        dev = r["dout"].astype(np.float64).transpose(2, 1, 0).reshape(u_pc, C_PAD)
        for j in range(u_pc):
            g = i * u_pc + j
            if types[j] == "D":
                d_rows.append(dev[j] / OUT_SCALE_D)
            else:
                d_rows.append(dev[j] / OUT_SCALE_A - shifts[g] * wn2_pad)
    d_s = np.asarray(d_rows)[:U, :C]
    return d_s, s_sym[:U]


def kernel(
    fc_weight,
    features_source,
    y_s,
    labels_source,
    Lambda,
    mean_source,
    mean_target,
    covariance_target,
):
    fc_weight = np.asarray(fc_weight, dtype=np.float32)
    y_s = np.asarray(y_s, dtype=np.float32)
    labels = np.asarray(labels_source).astype(np.int64)
    lam = float(np.asarray(Lambda))
    mean_source = np.asarray(mean_source, dtype=np.float32)
    mean_target = np.asarray(mean_target, dtype=np.float32)
    cov = np.asarray(covariance_target, dtype=np.float32)

    n = labels.shape[0]
    uniq, inv = np.unique(labels, return_inverse=True)

    d_s, s_sym = _device_dS(fc_weight, uniq, cov)

    # Cheap per-unique-label terms in float64 on host.
    w64 = fc_weight.astype(np.float64)
    wl = w64[uniq]                                         # [U, A]
    wv = np.einsum("uab,ub->ua", s_sym, wl)                # S_l @ W_l
    b = wv @ w64.T                                         # [U, C]
    s = np.einsum("ua,ua->u", wl, wv)                      # W_l S_l W_l^T
    quad = 0.5 * d_s - b + 0.5 * s[:, None]                # [U, C]

    d_mean = (mean_target - mean_source).astype(np.float64)[uniq]  # [U, A]
    g = d_mean @ w64.T                                     # [U, C]
    g_self = np.einsum("ua,ua->u", wl, d_mean)             # [U]

    aug = (
        y_s.astype(np.float64)
        + 0.5 * lam * quad[inv]
        + lam * (g[inv] - g_self[inv][:, None])
    )
    mx = aug.max(axis=1, keepdims=True)
    lse = mx[:, 0] + np.log(np.exp(aug - mx).sum(axis=1))
    nll = lse - aug[np.arange(n), labels]
    return np.array(nll.mean(), dtype=np.float32)


# revision 6
# speedup vs baseline: 1.1548x; 1.1548x over previous
"""Trainium2 Bass kernel for nn_Cls_Loss_42331197670001.

Reference computation (N=128 samples, C=345 classes, A=512 features):
    dataW[n,c,:] = W[c] - W[labels[n]]
    sigma2[n,c]  = Lambda * dataW[n,c] @ Sigma[labels[n]] @ dataW[n,c]^T
    dW_dMean[n,c]= dataW[n,c] . (mean_target-mean_source)[labels[n]]
    aug = y_s + 0.5*sigma2 + Lambda*dW_dMean ;  loss = mean softmax-CE(aug, labels)

Everything depends on the sample n only through its label l, so the heavy
quadratic form is computed once per *unique* label:
    (W_c - W_l) Sigma_l (W_c - W_l)^T = 0.5*d(l,c) - b(l,c) + 0.5*s(l)
    d(l,c) = W_c S_l W_c^T  with S_l = Sigma_l + Sigma_l^T
             <- the only O(C*A*A) term, on device
    b, s, mean-shift, softmax-CE <- tiny, host numpy in float64

Device computes d(l,c) with labels sharded over 8 cores (u_pc per core) in
one of two algebraically equal forms so the PSUM->SBUF reduce splits across
BOTH vector-capable engines (each PSUM element must cross DVE or ACT once;
that readback is the co-bottleneck with the PE):
  'D' labels (DVE):  P = W M_l   (M_l = upper-tri fold of S_l), then
                     d[c] = sum_b P[c,b] W[c,b] via one DVE
                     scalar_tensor_tensor straight out of PSUM (fp32 in1).
  'A' labels (ACT):  Q = W L_l   (L_l = chol(S_l + shift*I), lower-tri), then
                     d[c] = sum_b Q[c,b]^2 - shift*||W_c||^2 via one ACT
                     activation(Square, accum_out) straight out of PSUM
                     (the shift correction is host float64).
Both triangular forms stream 768 fp8-DoubleRow columns per (label, c-tile):
3 c-tiles of 128 classes, contraction over A=512 as two 256-row stationary
pairs.  Matmuls are emitted stationary-major per group of <=4 labels so
LDWEIGHTS pipelines behind dense matmul streams, and the 8 PSUM banks
ping-pong between the PE (current c-tile) and the consumers (previous).
sg DMAs are batched per group ([128, G*1536] contiguous per partition) to
amortize descriptor overhead; memset-fed junk matmuls warm the PE HAM
clock gate while the first DMAs land, and an early 1-element Square primes
the ACT table load off the critical path.
"""

import math
import sys

import numpy as np

try:
    import concourse.bass as bass
except ImportError:  # harness runs from a bare directory
    sys.path.insert(0, "/opt/trn_rl_repo")
    import concourse.bass as bass

import ml_dtypes

import concourse.mybir as mybir
import concourse.tile as tile
from concourse import bacc
from concourse.bass import ts
from concourse.bass_utils import run_bass_kernel_spmd

N_CORES = 8
A = 512          # feature dim
C = 345          # class count
C_PAD = 384      # 3 * 128
C_TILES = 3

W_SCALE = 16.0
S_SCALE = 32.0
SL_SCALE = 64.0
OUT_SCALE_D = W_SCALE * S_SCALE
OUT_SCALE_A = (W_SCALE * SL_SCALE) ** 2

FP8 = mybir.dt.float8e4
BF16 = mybir.dt.bfloat16
F32 = mybir.dt.float32
FP8_NP = ml_dtypes.float8_e4m3

MULT = mybir.AluOpType.mult
DR = mybir.MatmulPerfMode.DoubleRow
SQUARE = mybir.ActivationFunctionType.Square


def make_groups(u_pc: int) -> list[list[int]]:
    """Slot indices grouped for the PSUM ping-pong: a small first group so
    the PE starts early, then groups of 4."""
    if u_pc <= 2:
        return [list(range(u_pc))]
    groups = [[0, 1]]
    i = 2
    while i < u_pc:
        groups.append(list(range(i, min(i + 4, u_pc))))
        i += 4
    return groups


def slot_types(u_pc: int) -> list[str]:
    """'D' (DVE consumes) or 'A' (ACT consumes) per slot.  Groups are kept
    locally balanced (bank recycling tracks both consumers) while the
    global ratio lands near the measured DVE:ACT throughput (~24:18)."""
    groups = make_groups(u_pc)
    types = ["A"] * u_pc
    n_d_target = round(u_pc * 24.0 / 42.0)
    n_d = 0
    for gi, g in enumerate(groups):
        n = len(g)
        nd = (n + 1) // 2 if n > 1 else 1
        # hand the remainder D's to the last group
        if gi == len(groups) - 1:
            nd = max(0, min(n, n_d_target - n_d))
        for k, s in enumerate(g):
            types[s] = "D" if k < nd else "A"
        n_d += nd
    return types


def build_nc(u_pc: int) -> bass.Bass:
    """Per core: u_pc labels; dout[p, t, j] = d-value for class c = 128*t + p
    of slot j (scaled by OUT_SCALE_D or OUT_SCALE_A depending on slot type)."""
    groups = make_groups(u_pc)
    types = slot_types(u_pc)

    nc = bacc.Bacc()
    wt8 = nc.dram_tensor("wt8", [128, 4, C_PAD], FP8, kind="ExternalInput")
    w32 = nc.dram_tensor("w32", [128, C_TILES, A], F32, kind="ExternalInput")
    # per slot j: [:, j, :, 0:512]  = 256 contraction rows (a-pair) x all cols
    #            [:, j, :, 512:768] = the other 256 rows x their 256 cols
    sg = nc.dram_tensor("sg", [128, u_pc, 2, 768], FP8, kind="ExternalInput")
    dout = nc.dram_tensor("dout", [128, C_TILES, u_pc], F32, kind="ExternalOutput")

    n4 = sum(1 for g in groups if len(g) == 4)
    with tile.TileContext(nc) as tc:
        with (
            tc.tile_pool(name="singles", bufs=1) as singles,
            tc.tile_pool(name="sgp", bufs=max(1, n4)) as sgpool,
            tc.tile_pool(name="scr", bufs=8) as scrpool,
            tc.tile_pool(name="psum", bufs=8, space="PSUM") as ppool,
        ):
            # All input DMAs upfront so descriptor generation on the queues
            # never sits behind mid-kernel semaphore traffic.  w32 rides the
            # scalar hwdge queue (only consumed once STTs begin).
            wt8_sb = singles.tile([128, 4, C_PAD], FP8)
            nc.sync.dma_start(out=wt8_sb[:], in_=wt8[:])
            gts = []
            for slots in groups:
                g0, gn = slots[0], len(slots)
                if gn == 4:
                    gt = sgpool.tile([128, gn, 2, 768], FP8, tag="sg",
                                     name=f"sg{g0}")
                else:
                    gt = singles.tile([128, gn, 2, 768], FP8, name=f"sgs{g0}")
                nc.sync.dma_start(out=gt[:], in_=sg[:, g0 : g0 + gn])
                gts.append(gt)
            w32_sb = singles.tile([128, C_TILES, A], F32)
            nc.scalar.dma_start(out=w32_sb[:], in_=w32[:])
            d_all = singles.tile([128, C_TILES, u_pc], F32)

            # Prime the ACT Square table load off the critical path.
            prime_o = singles.tile([128, 1], F32)
            nc.scalar.activation(out=prime_o[:], in_=w32_sb[:, 0, 0:1],
                                 func=SQUARE)

            for gi, slots in enumerate(groups):
                g0, gn = slots[0], len(slots)
                gt = gts[gi]
                d_loc = [k for k in range(gn) if types[slots[k]] == "D"]
                a_loc = [k for k in range(gn) if types[slots[k]] == "A"]
                for t in range(C_TILES):
                    lhs01 = wt8_sb[:, 0:2, ts(t, 128)]
                    lhs23 = wt8_sb[:, 2:4, ts(t, 128)]
                    ps = [
                        ppool.tile([128, A], F32, tag="ps", name=f"ps{k}")
                        for k in range(gn)
                    ]
                    # pass 1 -- stationary a01: D-labels' first two matmuls
                    # (M rows 0:256 stream cols 0:512)
                    for k in d_loc:
                        nc.tensor.matmul(
                            ps[k][:, 0:256], lhsT=lhs01,
                            rhs=gt[:, k, :, 0:256],
                            start=True, stop=True, perf_mode=DR,
                        )
                        nc.tensor.matmul(
                            ps[k][:, 256:512], lhsT=lhs01,
                            rhs=gt[:, k, :, 256:512],
                            start=True, stop=False, perf_mode=DR,
                        )
                    # pass 2 -- stationary a23: D-labels finish (M rows
                    # 256:512 x cols 256:512); A-labels' first two matmuls
                    # (L rows 256:512 stream cols 0:512)
                    for k in d_loc:
                        nc.tensor.matmul(
                            ps[k][:, 256:512], lhsT=lhs23,
                            rhs=gt[:, k, :, 512:768],
                            start=False, stop=True, perf_mode=DR,
                        )
                    for k in a_loc:
                        nc.tensor.matmul(
                            ps[k][:, 0:256], lhsT=lhs23,
                            rhs=gt[:, k, :, 0:256],
                            start=True, stop=False, perf_mode=DR,
                        )
                        nc.tensor.matmul(
                            ps[k][:, 256:512], lhsT=lhs23,
                            rhs=gt[:, k, :, 256:512],
                            start=True, stop=True, perf_mode=DR,
                        )
                    # pass 3 -- stationary a01 again: A-labels finish
                    # (L rows 0:256 x cols 0:256)
                    for k in a_loc:
                        nc.tensor.matmul(
                            ps[k][:, 0:256], lhsT=lhs01,
                            rhs=gt[:, k, :, 512:768],
                            start=False, stop=True, perf_mode=DR,
                        )
                    # consumers: one engine pass per PSUM tile
                    for k in d_loc:
                        j = slots[k]
                        o = scrpool.tile([128, A], BF16, tag="o")
                        nc.vector.scalar_tensor_tensor(
                            out=o[:], in0=ps[k][:], scalar=1.0,
                            in1=w32_sb[:, t, :], op0=MULT, op1=MULT,
                            accum_out=d_all[:, t, j : j + 1],
                        )
                    for k in a_loc:
                        j = slots[k]
                        o = scrpool.tile([128, A], BF16, tag="oa")
                        nc.scalar.activation(
                            out=o[:], in_=ps[k][:], func=SQUARE,
                            accum_out=d_all[:, t, j : j + 1],
                        )
                # this group's d columns are final -- stream them out now
                nc.sync.dma_start(
                    out=dout[:, :, g0 : g0 + gn], in_=d_all[:, :, g0 : g0 + gn]
                )
    nc.compile()
    return nc


def _q8(x: np.ndarray) -> np.ndarray:
    return np.clip(x, -240.0, 240.0).astype(FP8_NP)


def _pack_tri(rows_hi: np.ndarray, corner: np.ndarray, scale: float) -> np.ndarray:
    """[256, 512] full-width rows + [256, 256] corner -> [128, 2, 768] fp8
    in DoubleRow layout (contraction row = 128*ko + ki)."""
    out = np.empty((128, 2, 768), FP8_NP)
    out[:, :, 0:512] = _q8(
        rows_hi.reshape(2, 128, A).transpose(1, 0, 2) * scale
    )
    out[:, :, 512:768] = _q8(
        corner.reshape(2, 128, 256).transpose(1, 0, 2) * scale
    )
    return out


def host_pack(fc_weight: np.ndarray, lab_pad: np.ndarray, cov: np.ndarray):
    """Build device inputs.  Returns (wt8, w32, sg[all cores], shifts,
    S_sym) where sg is [n_slots, 128, 2, 768] (slot-major; reshaped to the
    [128, u_pc, 2, 768] per-core layout by the caller)."""
    n_lab = len(lab_pad)
    u_pc = n_lab // N_CORES
    types = slot_types(u_pc)

    w_pad = np.zeros((C_PAD, A), np.float32)
    w_pad[:C] = fc_weight
    wt = np.ascontiguousarray(w_pad.T.reshape(4, 128, C_PAD).transpose(1, 0, 2))
    wt8 = _q8(wt * W_SCALE)
    w32 = np.ascontiguousarray(w_pad.reshape(C_TILES, 128, A).transpose(1, 0, 2))

    sgath = cov[lab_pad].astype(np.float64)
    s_sym = sgath + sgath.transpose(0, 2, 1)
    sig = s_sym.std(axis=(1, 2))
    eye = np.eye(A)

    sg = np.empty((n_lab, 128, 2, 768), FP8_NP)
    shifts = np.zeros(n_lab)
    for i in range(n_lab):
        if types[i % u_pc] == "D":
            m = np.triu(s_sym[i]) + np.triu(s_sym[i], 1)
            sg[i] = _pack_tri(m[0:256, :], m[256:512, 256:512], S_SCALE)
        else:
            c = 1.12 * 2.0 * math.sqrt(A) * sig[i]
            while True:
                try:
                    low = np.linalg.cholesky(s_sym[i] + c * eye)
                    break
                except np.linalg.LinAlgError:
                    c *= 1.15
            shifts[i] = c
            sg[i] = _pack_tri(low[256:512, :], low[0:256, 0:256], SL_SCALE)
    return wt8, w32, sg, shifts, s_sym


_NC_CACHE: dict[int, bass.Bass] = {}


def _device_dS(fc_weight, uniq, cov):
    """Run the Bass kernel on 8 cores; returns (d_S [U, C] float64, S_sym)."""
    U = len(uniq)
    u_pc = math.ceil(U / N_CORES)
    u_pad = u_pc * N_CORES
    lab_pad = np.concatenate([uniq, np.full(u_pad - U, uniq[0], dtype=uniq.dtype)])
    wt8, w32, sg, shifts, s_sym = host_pack(fc_weight, lab_pad, cov)

    if u_pc not in _NC_CACHE:
        _NC_CACHE[u_pc] = build_nc(u_pc)
    nc = _NC_CACHE[u_pc]

    in_maps = [
        {
            "wt8": wt8,
            "w32": w32,
            # [u_pc, 128, 2, 768] -> [128, u_pc, 2, 768]
            "sg": np.ascontiguousarray(
                sg[i * u_pc : (i + 1) * u_pc].transpose(1, 0, 2, 3)
            ),
        }
        for i in range(N_CORES)
    ]
    res = run_bass_kernel_spmd(nc, in_maps, core_ids=list(range(N_CORES)))

    types = slot_types(u_pc)
    wn2 = (fc_weight.astype(np.float64) ** 2).sum(axis=1)  # [C]
    wn2_pad = np.zeros(C_PAD)
    wn2_pad[:C] = wn2
    d_rows = []
    for i, r in enumerate(res.results):
        dev = r["dout"].astype(np.float64).transpose(2, 1, 0).reshape(u_pc, C_PAD)
        for j in range(u_pc):
            g = i * u_pc + j
            if types[j] == "D":
                d_rows.append(dev[j] / OUT_SCALE_D)
            else:
                d_rows.append(dev[j] / OUT_SCALE_A - shifts[g] * wn2_pad)
    d_s = np.asarray(d_rows)[:U, :C]
    return d_s, s_sym[:U]


def kernel(
    fc_weight,
    features_source,
    y_s,
    labels_source,
    Lambda,
    mean_source,
    mean_target,
    covariance_target,
):
    fc_weight = np.asarray(fc_weight, dtype=np.float32)
    y_s = np.asarray(y_s, dtype=np.float32)
    labels = np.asarray(labels_source).astype(np.int64)
    lam = float(np.asarray(Lambda))
    mean_source = np.asarray(mean_source, dtype=np.float32)
    mean_target = np.asarray(mean_target, dtype=np.float32)
    cov = np.asarray(covariance_target, dtype=np.float32)

    n = labels.shape[0]
    uniq, inv = np.unique(labels, return_inverse=True)

    d_s, s_sym = _device_dS(fc_weight, uniq, cov)

    # Cheap per-unique-label terms in float64 on host.
    w64 = fc_weight.astype(np.float64)
    wl = w64[uniq]                                         # [U, A]
    wv = np.einsum("uab,ub->ua", s_sym, wl)                # S_l @ W_l
    b = wv @ w64.T                                         # [U, C]
    s = np.einsum("ua,ua->u", wl, wv)                      # W_l S_l W_l^T
    quad = 0.5 * d_s - b + 0.5 * s[:, None]                # [U, C]

    d_mean = (mean_target - mean_source).astype(np.float64)[uniq]  # [U, A]
    g = d_mean @ w64.T                                     # [U, C]
    g_self = np.einsum("ua,ua->u", wl, d_mean)             # [U]

    aug = (
        y_s.astype(np.float64)
        + 0.5 * lam * quad[inv]
        + lam * (g[inv] - g_self[inv][:, None])
    )
    mx = aug.max(axis=1, keepdims=True)
    lse = mx[:, 0] + np.log(np.exp(aug - mx).sum(axis=1))
    nll = lse - aug[np.arange(n), labels]
    return np.array(nll.mean(), dtype=np.float32)


# revision 10
# speedup vs baseline: 1.1887x; 1.0294x over previous
"""Trainium2 Bass kernel for nn_Cls_Loss_42331197670001.

Reference computation (N=128 samples, C=345 classes, A=512 features):
    dataW[n,c,:] = W[c] - W[labels[n]]
    sigma2[n,c]  = Lambda * dataW[n,c] @ Sigma[labels[n]] @ dataW[n,c]^T
    dW_dMean[n,c]= dataW[n,c] . (mean_target-mean_source)[labels[n]]
    aug = y_s + 0.5*sigma2 + Lambda*dW_dMean ;  loss = mean softmax-CE(aug, labels)

Everything depends on the sample n only through its label l, so the heavy
quadratic form is computed once per *unique* label:
    (W_c - W_l) Sigma_l (W_c - W_l)^T = 0.5*d(l,c) - b(l,c) + 0.5*s(l)
    d(l,c) = W_c S_l W_c^T  with S_l = Sigma_l + Sigma_l^T
             <- the only O(C*A*A) term, on device
    b, s, mean-shift, softmax-CE <- tiny, host numpy in float64

Device computes d(l,c) with labels sharded over 8 cores (u_pc per core) in
one of two algebraically equal forms so the PSUM->SBUF reduce splits across
BOTH vector-capable engines (each PSUM element must cross DVE or ACT once;
that readback is the co-bottleneck with the PE):
  'D' labels (DVE):  P = W M_l   (M_l = upper-tri fold of S_l), then
                     d[c] = sum_b P[c,b] W[c,b] via one DVE
                     scalar_tensor_tensor straight out of PSUM (fp32 in1).
  'A' labels (ACT):  Q = W L_l   (L_l = chol(S_l + shift*I), lower-tri), then
                     d[c] = sum_b Q[c,b]^2 - shift*||W_c||^2 via one ACT
                     activation(Square, accum_out) straight out of PSUM
                     (the shift correction is host float64).
Both triangular forms stream 768 fp8-DoubleRow columns per (label, c-tile):
3 c-tiles of 128 classes, contraction over A=512 as two 256-row stationary
pairs.  Matmuls are emitted stationary-major per group of <=4 labels so
LDWEIGHTS pipelines behind dense matmul streams, and the 8 PSUM banks
ping-pong between the PE (current c-tile) and the consumers (previous).
sg DMAs are batched per group ([128, G*1536] contiguous per partition) to
amortize descriptor overhead; memset-fed junk matmuls warm the PE HAM
clock gate while the first DMAs land, and an early 1-element Square primes
the ACT table load off the critical path.
"""

import math
import sys

import numpy as np

try:
    import concourse.bass as bass
except ImportError:  # harness runs from a bare directory
    sys.path.insert(0, "/opt/trn_rl_repo")
    import concourse.bass as bass

import ml_dtypes

import concourse.mybir as mybir
import concourse.tile as tile
from concourse import bacc
from concourse.bass import ts
from concourse.bass_utils import run_bass_kernel_spmd

N_CORES = 8
A = 512          # feature dim
C = 345          # class count
C_PAD = 384      # 3 * 128
C_TILES = 3

W_SCALE = 16.0
S_SCALE = 32.0
SL_SCALE = 64.0
OUT_SCALE_D = W_SCALE * S_SCALE
OUT_SCALE_A = (W_SCALE * SL_SCALE) ** 2

FP8 = mybir.dt.float8e4
BF16 = mybir.dt.bfloat16
F32 = mybir.dt.float32
FP8_NP = ml_dtypes.float8_e4m3

MULT = mybir.AluOpType.mult
DR = mybir.MatmulPerfMode.DoubleRow
SQUARE = mybir.ActivationFunctionType.Square


def make_groups(u_pc: int) -> list[list[int]]:
    """Slot indices grouped for the PSUM ping-pong: a small first group so
    the PE starts early, then groups of 4."""
    if u_pc <= 2:
        return [list(range(u_pc))]
    groups = [[0, 1]]
    i = 2
    while i < u_pc:
        groups.append(list(range(i, min(i + 4, u_pc))))
        i += 4
    return groups


def slot_types(u_pc: int) -> list[str]:
    """'D' (DVE consumes) or 'A' (ACT consumes) per slot.  Groups are kept
    locally balanced (bank recycling tracks both consumers) while the
    global ratio lands near the measured DVE:ACT throughput (~24:18)."""
    groups = make_groups(u_pc)
    types = ["A"] * u_pc
    n_d_target = round(u_pc * 24.0 / 42.0)
    n_d = 0
    for gi, g in enumerate(groups):
        n = len(g)
        nd = (n + 1) // 2 if n > 1 else 1
        # hand the remainder D's to the last group
        if gi == len(groups) - 1:
            nd = max(0, min(n, n_d_target - n_d))
        for k, s in enumerate(g):
            types[s] = "D" if k < nd else "A"
        n_d += nd
    return types


def build_nc(u_pc: int) -> bass.Bass:
    """Per core: u_pc labels; dout[p, t, j] = d-value for class c = 128*t + p
    of slot j (scaled by OUT_SCALE_D or OUT_SCALE_A depending on slot type)."""
    groups = make_groups(u_pc)
    types = slot_types(u_pc)

    nc = bacc.Bacc()
    wt8 = nc.dram_tensor("wt8", [128, 4, C_PAD], FP8, kind="ExternalInput")
    w32 = nc.dram_tensor("w32", [128, C_TILES, A], F32, kind="ExternalInput")
    # per slot j: [:, j, :, 0:512]  = 256 contraction rows (a-pair) x all cols
    #            [:, j, :, 512:768] = the other 256 rows x their 256 cols
    sg = nc.dram_tensor("sg", [128, u_pc, 2, 768], FP8, kind="ExternalInput")
    dout = nc.dram_tensor("dout", [128, C_TILES, u_pc], F32, kind="ExternalOutput")

    n4 = sum(1 for g in groups if len(g) == 4)
    with tile.TileContext(nc) as tc:
        with (
            tc.tile_pool(name="singles", bufs=1) as singles,
            tc.tile_pool(name="sgp", bufs=max(1, n4)) as sgpool,
            tc.tile_pool(name="scr", bufs=8) as scrpool,
            tc.tile_pool(name="psum", bufs=8, space="PSUM") as ppool,
        ):
            # All input DMAs upfront so descriptor generation on the queues
            # never sits behind mid-kernel semaphore traffic.  w32 rides the
            # scalar hwdge queue (only consumed once STTs begin).
            wt8_sb = singles.tile([128, 4, C_PAD], FP8)
            nc.sync.dma_start(out=wt8_sb[:], in_=wt8[:])
            gts = []
            for slots in groups:
                g0, gn = slots[0], len(slots)
                if gn == 4:
                    gt = sgpool.tile([128, gn, 2, 768], FP8, tag="sg",
                                     name=f"sg{g0}")
                else:
                    gt = singles.tile([128, gn, 2, 768], FP8, name=f"sgs{g0}")
                nc.sync.dma_start(out=gt[:], in_=sg[:, g0 : g0 + gn])
                gts.append(gt)
            w32_sb = singles.tile([128, C_TILES, A], F32)
            nc.scalar.dma_start(out=w32_sb[:], in_=w32[:])
            d_all = singles.tile([128, C_TILES, u_pc], F32)

            # Prime the ACT Square table load off the critical path.
            prime_o = singles.tile([128, 1], F32)
            nc.scalar.activation(out=prime_o[:], in_=w32_sb[:, 0, 0:1],
                                 func=SQUARE)

            # Junk matmuls on the (small, first-landing) weight tile warm the
            # PE HAM clock gate while the sg tiles stream in.
            junkp = ppool.tile([128, A], F32, tag="ps")
            for _ in range(8):
                nc.tensor.matmul(
                    junkp[:, 0:256], lhsT=wt8_sb[:, 0:2, 0:128],
                    rhs=wt8_sb[:, 0:2, 0:256],
                    start=True, stop=True, perf_mode=DR,
                )

            for gi, slots in enumerate(groups):
                g0, gn = slots[0], len(slots)
                gt = gts[gi]
                d_loc = [k for k in range(gn) if types[slots[k]] == "D"]
                a_loc = [k for k in range(gn) if types[slots[k]] == "A"]
                for t in range(C_TILES):
                    lhs01 = wt8_sb[:, 0:2, ts(t, 128)]
                    lhs23 = wt8_sb[:, 2:4, ts(t, 128)]
                    ps = [
                        ppool.tile([128, A], F32, tag="ps", name=f"ps{k}")
                        for k in range(gn)
                    ]
                    # pass 1 -- stationary a01: D-labels' full-width matmul
                    # (M rows 0:256 stream cols 0:512)
                    for k in d_loc:
                        nc.tensor.matmul(
                            ps[k][:], lhsT=lhs01,
                            rhs=gt[:, k, :, 0:512],
                            start=True, stop=False, perf_mode=DR,
                            skip_group_check=True,
                        )
                    # pass 2 -- stationary a23: D-labels finish (M rows
                    # 256:512 x cols 256:512); A-labels' full-width matmul
                    # (L rows 256:512 stream cols 0:512)
                    for k in d_loc:
                        nc.tensor.matmul(
                            ps[k][:, 256:512], lhsT=lhs23,
                            rhs=gt[:, k, :, 512:768],
                            start=False, stop=True, perf_mode=DR,
                            skip_group_check=True,
                        )
                    for k in a_loc:
                        nc.tensor.matmul(
                            ps[k][:], lhsT=lhs23,
                            rhs=gt[:, k, :, 0:512],
                            start=True, stop=False, perf_mode=DR,
                            skip_group_check=True,
                        )
                    # pass 3 -- stationary a01 again: A-labels finish
                    # (L rows 0:256 x cols 0:256)
                    for k in a_loc:
                        nc.tensor.matmul(
                            ps[k][:, 0:256], lhsT=lhs01,
                            rhs=gt[:, k, :, 512:768],
                            start=False, stop=True, perf_mode=DR,
                            skip_group_check=True,
                        )
                    # consumers: one engine pass per PSUM tile
                    for k in d_loc:
                        j = slots[k]
                        o = scrpool.tile([128, A], BF16, tag="o")
                        nc.vector.scalar_tensor_tensor(
                            out=o[:], in0=ps[k][:], scalar=1.0,
                            in1=w32_sb[:, t, :], op0=MULT, op1=MULT,
                            accum_out=d_all[:, t, j : j + 1],
                        )
                    for k in a_loc:
                        j = slots[k]
                        o = scrpool.tile([128, A], BF16, tag="oa")
                        nc.scalar.activation(
                            out=o[:], in_=ps[k][:], func=SQUARE,
                            accum_out=d_all[:, t, j : j + 1],
                        )
                # this group's d columns are final -- stream them out now
                # (scalar hwdge queue: its wait order matches ACT's natural
                # program order, and it keeps the sync queue free to relay
                # input-DMA completions)
                nc.scalar.dma_start(
                    out=dout[:, :, g0 : g0 + gn], in_=d_all[:, :, g0 : g0 + gn]
                )
    nc.compile()
    return nc


def _q8(x: np.ndarray) -> np.ndarray:
    return np.clip(x, -240.0, 240.0).astype(FP8_NP)


def _pack_tri(rows_hi: np.ndarray, corner: np.ndarray, scale: float) -> np.ndarray:
    """[256, 512] full-width rows + [256, 256] corner -> [128, 2, 768] fp8
    in DoubleRow layout (contraction row = 128*ko + ki)."""
    out = np.empty((128, 2, 768), FP8_NP)
    out[:, :, 0:512] = _q8(
        rows_hi.reshape(2, 128, A).transpose(1, 0, 2) * scale
    )
    out[:, :, 512:768] = _q8(
        corner.reshape(2, 128, 256).transpose(1, 0, 2) * scale
    )
    return out


def host_pack(fc_weight: np.ndarray, lab_pad: np.ndarray, cov: np.ndarray):
    """Build device inputs.  Returns (wt8, w32, sg[all cores], shifts,
    S_sym) where sg is [n_slots, 128, 2, 768] (slot-major; reshaped to the
    [128, u_pc, 2, 768] per-core layout by the caller)."""
    n_lab = len(lab_pad)
    u_pc = n_lab // N_CORES
    types = slot_types(u_pc)

    w_pad = np.zeros((C_PAD, A), np.float32)
    w_pad[:C] = fc_weight
    wt = np.ascontiguousarray(w_pad.T.reshape(4, 128, C_PAD).transpose(1, 0, 2))
    wt8 = _q8(wt * W_SCALE)
    w32 = np.ascontiguousarray(w_pad.reshape(C_TILES, 128, A).transpose(1, 0, 2))

    sgath = cov[lab_pad].astype(np.float64)
    s_sym = sgath + sgath.transpose(0, 2, 1)
    sig = s_sym.std(axis=(1, 2))
    eye = np.eye(A)

    sg = np.empty((n_lab, 128, 2, 768), FP8_NP)
    shifts = np.zeros(n_lab)
    for i in range(n_lab):
        if types[i % u_pc] == "D":
            m = np.triu(s_sym[i]) + np.triu(s_sym[i], 1)
            sg[i] = _pack_tri(m[0:256, :], m[256:512, 256:512], S_SCALE)
        else:
            c = 1.12 * 2.0 * math.sqrt(A) * sig[i]
            while True:
                try:
                    low = np.linalg.cholesky(s_sym[i] + c * eye)
                    break
                except np.linalg.LinAlgError:
                    c *= 1.15
            shifts[i] = c
            sg[i] = _pack_tri(low[256:512, :], low[0:256, 0:256], SL_SCALE)
    return wt8, w32, sg, shifts, s_sym


_NC_CACHE: dict[int, bass.Bass] = {}


def _device_dS(fc_weight, uniq, cov):
    """Run the Bass kernel on 8 cores; returns (d_S [U, C] float64, S_sym)."""
    U = len(uniq)
    u_pc = math.ceil(U / N_CORES)
    u_pad = u_pc * N_CORES
    lab_pad = np.concatenate([uniq, np.full(u_pad - U, uniq[0], dtype=uniq.dtype)])
    wt8, w32, sg, shifts, s_sym = host_pack(fc_weight, lab_pad, cov)

    if u_pc not in _NC_CACHE:
        _NC_CACHE[u_pc] = build_nc(u_pc)
    nc = _NC_CACHE[u_pc]

    in_maps = [
        {
            "wt8": wt8,
            "w32": w32,
            # [u_pc, 128, 2, 768] -> [128, u_pc, 2, 768]
            "sg": np.ascontiguousarray(
                sg[i * u_pc : (i + 1) * u_pc].transpose(1, 0, 2, 3)
            ),
        }
        for i in range(N_CORES)
    ]
    res = run_bass_kernel_spmd(nc, in_maps, core_ids=list(range(N_CORES)))

    types = slot_types(u_pc)
    wn2 = (fc_weight.astype(np.float64) ** 2).sum(axis=1)  # [C]
    wn2_pad = np.zeros(C_PAD)
    wn2_pad[:C] = wn2
    d_rows = []
    for i, r in enumerate(res.results):
        dev = r["dout"].astype(np.float64).transpose(2, 1, 0).reshape(u_pc, C_PAD)
        for j in range(u_pc):
            g = i * u_pc + j
            if types[j] == "D":
                d_rows.append(dev[j] / OUT_SCALE_D)
            else:
                d_rows.append(dev[j] / OUT_SCALE_A - shifts[g] * wn2_pad)
    d_s = np.asarray(d_rows)[:U, :C]
    return d_s, s_sym[:U]


def kernel(
    fc_weight,
    features_source,
    y_s,
    labels_source,
    Lambda,
    mean_source,
    mean_target,
    covariance_target,
):
    fc_weight = np.asarray(fc_weight, dtype=np.float32)
    y_s = np.asarray(y_s, dtype=np.float32)
    labels = np.asarray(labels_source).astype(np.int64)
    lam = float(np.asarray(Lambda))
    mean_source = np.asarray(mean_source, dtype=np.float32)
    mean_target = np.asarray(mean_target, dtype=np.float32)
    cov = np.asarray(covariance_target, dtype=np.float32)

    n = labels.shape[0]
    uniq, inv = np.unique(labels, return_inverse=True)

    d_s, s_sym = _device_dS(fc_weight, uniq, cov)

    # Cheap per-unique-label terms in float64 on host.
    w64 = fc_weight.astype(np.float64)
    wl = w64[uniq]                                         # [U, A]
    wv = np.einsum("uab,ub->ua", s_sym, wl)                # S_l @ W_l
    b = wv @ w64.T                                         # [U, C]
    s = np.einsum("ua,ua->u", wl, wv)                      # W_l S_l W_l^T
    quad = 0.5 * d_s - b + 0.5 * s[:, None]                # [U, C]

    d_mean = (mean_target - mean_source).astype(np.float64)[uniq]  # [U, A]
    g = d_mean @ w64.T                                     # [U, C]
    g_self = np.einsum("ua,ua->u", wl, d_mean)             # [U]

    aug = (
        y_s.astype(np.float64)
        + 0.5 * lam * quad[inv]
        + lam * (g[inv] - g_self[inv][:, None])
    )
    mx = aug.max(axis=1, keepdims=True)
    lse = mx[:, 0] + np.log(np.exp(aug - mx).sum(axis=1))
    nll = lse - aug[np.arange(n), labels]
    return np.array(nll.mean(), dtype=np.float32)
